# revision 1
# baseline (speedup 1.0000x reference)
"""Trainium2 Bass kernel for nn_MultiHeadDotProductAttention_24756191494231.

Masked (toeplitz-structured) linear attention:
    q = relu(query/8); k = relu(key)
    attn = (q @ k^T) * |toeplitz_mask| ; attn /= attn.sum(-1) ; out = attn @ v

Sharding: 8 cores = 2 batch-groups (4 batches) x 4 head-groups (3 heads).
Each core computes 12 (batch, head) pairs.

Device pipeline per (head, batch):
  S^T[k,l] = K'^T.T @ Q'^T      (bf16 matmuls, PSUM f32, k-chunks of <=121)
  A[k,l]   = S^T * |mask^T|     (tensor_tensor; mask read from a
                                 shift-replicated params tile via a strided AP
                                 -- the toeplitz gather becomes pure layout)
  O[l,:]   = A.T @ [V|1]        (bf16; ones column gives Z = row-sum)
  out      = O[:, :64] / Z      (reciprocal + broadcast multiply)

The mask operand tile ("mop") holds |params| shifted by s(r) = r%24 + 48*(r//24)
per partition; one AP with dims [[48,24],[1,24]] then reads mask^T rows for a
whole 120-row chunk. The shifts are materialized by a single DRAM->DRAM DMA
whose output access pattern is a parallelogram (affine in flat DRAM space).

Mask application is split across engines per k-chunk: chunk 0 goes straight
through DVE from PSUM; chunks 1-4 are copied PSUM->SBUF(bf16) on ScalarE, then
multiplied in-place in bf16 2x mode on DVE (chunks 1,3,4) or GpSimd (chunk 2).
"""
import sys

for _p in ("/opt/trn_rl_repo", "/root/.axon_site/_ro/trn_rl_repo"):
    if _p not in sys.path:
        sys.path.insert(0, _p)

import numpy as np
import ml_dtypes

NBX = NBY = 24
B, H, D = 8, 12, 64
L = NBX * NBY + 1          # 577
LP = 578                   # A-tile chunk stride (even => bf16 runs 4B-aligned)
NB = 4                     # batches per core
NH = 3                     # heads per core
CNT = [121, 120, 120, 120, 96]       # k-chunk sizes (CLS + 24-aligned grid)
KS = [0, 121, 241, 361, 481]         # k-chunk starts (in l index)
LW = [128, 128, 128, 128, 65]        # l-chunk sizes
MOPW = 2520                          # stage row width (2304 + max shift 215 + 1)
CLSW = 1128                          # mop_cls width (f in [1176, 2304))
MAINW = 1848                         # mop_main width (f in [216, 2064))

_CACHE = {}


def _split_excess_waits(nc):
    """Walrus in this toolchain accepts at most ONE sync-wait per instruction
    (zero on Pool-engine ops). Move excess waits onto same-engine
    InstEventSemaphore instructions inserted immediately before the offending
    instruction; engines execute in order, so semantics are unchanged."""
    import concourse.mybir as mb
    ctr = 0
    f = nc.m.functions[0]
    for bb in f.blocks:
        insts = list(bb.instructions)
        out = []
        changed = False
        for inst in insts:
            si = inst.sync_info
            keep = 0 if inst.engine == mb.EngineType.Pool else 1
            if si is not None and len(si.on_wait) > keep:
                waits = list(si.on_wait)
                moved = waits[:-keep] if keep else waits
                kept = waits[-keep:] if keep else []
                for w in moved:
                    ctr += 1
                    ev = mb.InstEventSemaphore(
                        name=f"zz_waitsplit_{ctr}", ins=[], outs=[])
                    ev.engine = inst.engine
                    ev.sync_info = mb.SyncInfo(on_wait=[w], on_update=[])
                    out.append(ev)
                inst.sync_info = mb.SyncInfo(
                    on_wait=kept, on_update=list(si.on_update))
                changed = True
            out.append(inst)
        if changed:
            bb.instructions = out


def _build_bass():
    import concourse.bass as bass
    import concourse.mybir as mybir
    from concourse.bass_types import AP
    from concourse.tile import TileContext

    F32 = mybir.dt.float32
    BF16 = mybir.dt.bfloat16
    Alu = mybir.AluOpType
    Act = mybir.ActivationFunctionType

    nc = bass.Bass("TRN2")
    qkv_d = nc.dram_tensor("qkv", (NH, 128, 4 * L + NB * 325), F32,
                           kind="ExternalInput")
    prm_d = nc.dram_tensor("prm", (128, 54), F32, kind="ExternalInput")
    ones_d = nc.dram_tensor("ones1", (1, CLSW), BF16, kind="ExternalInput")
    o_d = nc.dram_tensor("o", (NH, NB, 128, 320), F32, kind="ExternalOutput")

    with TileContext(nc) as tc:
        with (
            tc.tile_pool(name="sb", bufs=2) as sb,
            tc.tile_pool(name="sb3", bufs=3) as sb3,
            tc.tile_pool(name="sb1", bufs=1) as sb1,
            tc.tile_pool(name="ps", bufs=3, space="PSUM") as ps,
            tc.tile_pool(name="ps_o", bufs=2, space="PSUM") as ps_o,
            tc.tile_pool(name="dram", bufs=1, space="DRAM") as dr,
        ):
            # ---- |params| -> DRAM scratch (flat, per-head contiguous) ----
            prm_sb = sb1.tile([128, 54], F32)
            nc.sync.dma_start(prm_sb, prm_d[:, :])
            prm_abs = sb1.tile([128, 54], F32)
            nc.scalar.activation(prm_abs, prm_sb, Act.Abs)
            scratch = dr.tile([NH * 2304], F32, tag="scratch")
            nc.sync.dma_start(AP(scratch.tensor, 0, [[54, 128], [1, 54]]), prm_abs)

            def load_head(h):
                # ---- mask operand tiles ----
                stage = dr.tile([121 * MOPW + 64], BF16, tag=f"stage{h}")
                out_ap = AP(stage.tensor, MOPW,
                            [[24 * MOPW + 48, 5], [MOPW + 1, 24], [1, 2304]])
                in_ap = AP(scratch.tensor, h * 2304, [[0, 5], [0, 24], [1, 2304]])
                nc.gpsimd.dma_start(out_ap, in_ap)   # SWDGE: cast + step-0 src

                # mop_cls[p, y] = stage[p, 1176 + y]; row 0 then overwritten
                # with ones (CLS mask row) via a direct HWDGE load
                mop_cls = sb.tile([121, CLSW], BF16, tag="mop_cls")
                nc.sync.dma_start(mop_cls,
                                  AP(stage.tensor, 1176, [[MOPW, 121], [1, CLSW]]))
                nc.sync.dma_start(mop_cls[0:1, :], ones_d[:, :])
                # mop_main[p, y] = stage[p + 1, 216 + y]
                mop_main = sb.tile([120, MAINW], BF16, tag="mop_main")
                nc.sync.dma_start(mop_main,
                                  AP(stage.tensor, MOPW + 216, [[MOPW, 120], [1, MAINW]]))

                # ---- Q|K|V in one SWDGE cast-load, then relu (4x mode) ----
                qkv_r = sb.tile([128, 4 * L + NB * 325], BF16, tag="qkv_r")
                nc.gpsimd.dma_start(qkv_r, qkv_d[h])
                qT_b = sb.tile([128, 2 * L], BF16, tag="qT_b")
                nc.vector.tensor_scalar(out=qT_b, in0=qkv_r[:, 0:2 * L],
                                        scalar1=0.125, scalar2=0.0,
                                        op0=Alu.mult, op1=Alu.max)
                kT_b = sb.tile([128, 2 * L], BF16, tag="kT_b")
                nc.vector.tensor_scalar(out=kT_b, in0=qkv_r[:, 2 * L:4 * L],
                                        scalar1=0.0, scalar2=None, op0=Alu.max)
                o_sb = sb.tile([128, NB * 320], F32, tag="o_sb")
                return dict(mop_cls=mop_cls, mop_main=mop_main, qkv=qkv_r,
                            qT=qT_b, kT=kT_b, o_sb=o_sb, h=h)

            def mask_chunk(R, c, s_ps, a_t):
                cnt = CNT[c]
                co = LP * c + 1               # a_t column of l=0 for chunk c
                if c == 0:
                    # direct: TT from PSUM + separate CLS-query col
                    nc.vector.tensor_copy(a_t[0:cnt, co:co + 1],
                                          s_ps[0:cnt, 0:1])
                    in1 = AP(R["mop_cls"].tensor, 0,
                             [[CLSW, 121], [48, 24], [1, 24]])
                    in0 = s_ps[0:cnt, 1:L].rearrange("p (i j) -> p i j", j=24)
                    outap = a_t[0:cnt, co + 1:co + L].rearrange(
                        "p (i j) -> p i j", j=24)
                    nc.vector.tensor_tensor(out=outap, in0=in0,
                                            in1=in1, op=Alu.mult)
                else:
                    # copy all 577 cols to bf16 on ScalarE, then
                    # multiply grid cols in place (2x bf16)
                    nc.scalar.activation(a_t[0:cnt, co:co + L],
                                         s_ps[0:cnt, 0:L], Act.Copy)
                    off = 48 * (20 - 5 * c)
                    in1 = AP(R["mop_main"].tensor, off,
                             [[MAINW, cnt], [48, 24], [1, 24]])
                    io = a_t[0:cnt, co + 1:co + L].rearrange(
                        "p (i j) -> p i j", j=24)
                    eng = nc.gpsimd if c == 2 else nc.vector
                    eng.tensor_tensor(out=io, in0=io, in1=in1, op=Alu.mult)

            def build_pair(R, b):
                # S^T matmuls + mask application for one (head, batch) pair
                pr = 64 * (b // 2)            # partition row of this batch pair
                xo = L * (b % 2)              # column offset within the pair
                a_t = sb3.tile([128, 5 * LP], BF16, tag="a_t")
                for c in range(5):
                    cnt = CNT[c]
                    s_ps = ps.tile([128, L], F32, tag="s_ps")
                    lhs = R["kT"][pr:pr + 64, xo + KS[c]:xo + KS[c] + cnt]
                    nc.tensor.matmul(s_ps[0:cnt, 0:512], lhs,
                                     R["qT"][pr:pr + 64, xo:xo + 512],
                                     start=True, stop=True)
                    nc.tensor.matmul(s_ps[0:cnt, 512:577], lhs,
                                     R["qT"][pr:pr + 64, xo + 512:xo + 577],
                                     start=True, stop=True)
                    mask_chunk(R, c, s_ps, a_t)
                return a_t

            def finish_pair(R, b, a_t):
                # A.T @ [V|1], normalize, and store when the head completes
                o_ps = ps_o.tile([128, 325], F32, tag="o_ps")
                for lc in range(5):
                    lw = LW[lc]
                    for c in range(5):
                        nc.tensor.matmul(
                            o_ps[0:lw, 65 * lc:65 * lc + 65],
                            a_t[0:CNT[c], LP * c + 1 + 128 * lc:LP * c + 1 + 128 * lc + lw],
                            R["qkv"][0:CNT[c], 4 * L + 325 * b + 65 * c:4 * L + 325 * b + 65 * c + 65],
                            start=(c == 0), stop=(c == 4))

                rz = sb.tile([128, 5], F32, tag="rz")
                zin = o_ps[:, :].rearrange("p (c d) -> p c d", d=65)[:, :, 64:65]
                nc.vector.reciprocal(rz[:, :].rearrange("p (c d) -> p c d", d=1), zin)
                in0 = o_ps[:, :].rearrange("p (c d) -> p c d", d=65)[:, :, 0:64]
                in1 = AP(rz.tensor, 0, [[5, 128], [1, 5], [0, 64]])
                nc.vector.tensor_tensor(
                    out=R["o_sb"][:, 320 * b:320 * b + 320].rearrange(
                        "p (c d) -> p c d", d=64),
                    in0=in0, in1=in1, op=Alu.mult)
                if b == NB - 1:
                    nc.sync.dma_start(
                        AP(o_d, R["h"] * NB * 128 * 320,
                           [[320, 128], [128 * 320, NB], [1, 320]]),
                        R["o_sb"])

            # software pipeline: masks for pair i overlap AV of pair i-1
            pending = None
            for h in range(NH):
                R = load_head(h)
                for b in range(NB):
                    a_t = build_pair(R, b)
                    if pending is not None:
                        finish_pair(*pending)
                    pending = (R, b, a_t)
            finish_pair(*pending)

    _split_excess_waits(nc)
    return nc


def _get_nc():
    if "nc" not in _CACHE:
        _CACHE["nc"] = _build_bass()
    return _CACHE["nc"]


def _host_shard(query, key, value, topological_params):
    """Build the 8 per-core input dicts (pure slicing / layout transforms)."""
    in_maps = []
    q = np.asarray(query, dtype=np.float32)
    k = np.asarray(key, dtype=np.float32)
    v = np.asarray(value, dtype=np.float32)
    p = np.asarray(topological_params, dtype=np.float32)
    ones1 = np.ones((1, CLSW), dtype=ml_dtypes.bfloat16)
    for u in range(2):            # batch group
        for g in range(4):        # head group
            bs = slice(4 * u, 4 * u + 4)
            hs = slice(3 * g, 3 * g + 3)

            def pack_T(x):
                # [4b, L, 3h, 64] -> [3h, 128p, 2*L]; p = d + 64*(b//2),
                # col = (b%2)*L + l
                t = x[bs, :, hs, :]                       # [4, L, 3, 64]
                t = t.transpose(2, 0, 3, 1)               # [3, 4, 64, L]
                t = t.reshape(3, 2, 2, 64, L)             # [3, bhi, blo, d, L]
                t = t.transpose(0, 1, 3, 2, 4)            # [3, bhi, d, blo, L]
                return np.ascontiguousarray(t.reshape(3, 128, 2 * L))

            vs = v[bs, :, hs, :]                          # [4, L, 3, 64]
            v_r = np.zeros((3, 128, NB, 5, 65), np.float32)
            for c in range(5):
                n = CNT[c]
                blk = vs[:, KS[c]:KS[c] + n].transpose(2, 1, 0, 3)
                v_r[:, :n, :, c, 0:64] = blk
                v_r[:, :n, :, c, 64] = 1.0
            qkv = np.concatenate(
                [pack_T(q), pack_T(k), v_r.reshape(3, 128, NB * 325)], axis=2)
            prm = np.ascontiguousarray(p[hs]).reshape(128, 54)
            in_maps.append({
                "qkv": np.ascontiguousarray(qkv),
                "prm": prm,
                "ones1": ones1,
            })
    return in_maps


def kernel(query, key, value, topological_params):
    from concourse import bass_utils
    nc = _get_nc()
    in_maps = _host_shard(query, key, value, topological_params)
    res = bass_utils.run_bass_kernel_spmd(nc, in_maps, core_ids=list(range(8)))
    out = np.empty((B, L, H, D), dtype=np.float32)
    for u in range(2):
        for g in range(4):
            o = res.results[4 * u + g]["o"]          # [3, 4, 128, 320]
            o = o.reshape(3, 4, 128, 5, 64)
            for lc in range(5):
                lw = LW[lc]
                blk = o[:, :, 0:lw, lc, :]           # [3, 4, lw, 64]
                out[4 * u:4 * u + 4, 128 * lc:128 * lc + lw, 3 * g:3 * g + 3, :] = \
                    blk.transpose(1, 2, 0, 3)
    return out



# revision 4
# speedup vs baseline: 1.2495x; 1.2495x over previous
"""Trainium2 Bass kernel for nn_MultiHeadDotProductAttention_24756191494231.

Masked (toeplitz-structured) linear attention:
    q = relu(query/8); k = relu(key)
    attn = (q @ k^T) * |toeplitz_mask| ; attn /= attn.sum(-1) ; out = attn @ v

Sharding: 8 cores = 2 batch-groups (4 batches) x 4 head-groups (3 heads).
Each core computes 12 (batch, head) pairs.

Host packs q/k (relu+scale folded, bf16, [d|b-hi, (b-lo, l)] layout), V with a
ones column per k-chunk (the AV matmul's 65th column then yields the row-sum
Z), and the full per-head |toeplitz| mask laid out exactly like the device A
tile ([k-in-chunk partition, 578*c + l] bf16).

Device pipeline per (head, batch) pair, software-pipelined one slot deep
(masks of pair i overlap the AV matmuls of pair i-1 on the PE):
  S^T[k,l] = K'^T.T @ Q'^T          (bf16 matmuls into PSUM, 5 k-chunks)
  A[k,l]   = S^T * msk              split across engines per chunk:
      c0      : DVE tensor_tensor straight from PSUM (fused copy+mask)
      c1..c3  : ScalarE copy PSUM->SBUF bf16, then the mask multiply in
                bf16 2x mode, column-split between DVE (head cols) and
                GpSimd (tail cols) -- one 3-chunk-strided DVE op
      c4      : head cols via ScalarE copy + DVE 2x mult, tail cols via
                DVE tensor_tensor from PSUM
  O[l,:]   = A.T @ [V|1]            (bf16; ones column gives Z = row-sum;
                                     c-order 0,4,1,2,3 so late masks fit)
  out      = O[:, :64] / Z          (DVE reciprocal + PSUM-sourced multiply,
                                     bf16 result DMA'd out per pair)
"""
import sys

for _p in ("/opt/trn_rl_repo", "/root/.axon_site/_ro/trn_rl_repo"):
    if _p not in sys.path:
        sys.path.insert(0, _p)

import numpy as np
import ml_dtypes

NBX = NBY = 24
B, H, D = 8, 12, 64
L = NBX * NBY + 1          # 577
LP = 578                   # chunk stride in A/msk tiles (even => 4B aligned)
NB = 4                     # batches per core
NH = 3                     # heads per core
CNT = [121, 120, 120, 120, 96]       # k-chunk sizes (CLS + 24-aligned grid)
KS = [0, 121, 241, 361, 481]         # k-chunk starts
LW = [128, 128, 128, 128, 65]        # l-chunk sizes for the AV matmuls

# tuning knobs (cols of the mask multiply given to each engine)
PC = 300                   # tail cols of chunks 1..3 multiplied on GpSimd
A4 = 284                   # head cols of chunk 4 via ScalarE copy + DVE mult
AV_ORDER = [0, 4, 1, 2, 3]  # accumulation order of AV k-chunks

_CACHE = {}


def _split_excess_waits(nc):
    """Walrus accepts at most ONE sync-wait per instruction (zero on
    Pool-engine ops). Move excess waits onto same-engine InstEventSemaphore
    instructions inserted immediately before the offending instruction."""
    import concourse.mybir as mb
    ctr = 0
    f = nc.m.functions[0]
    for bb in f.blocks:
        insts = list(bb.instructions)
        out = []
        changed = False
        for inst in insts:
            si = inst.sync_info
            keep = 0 if inst.engine == mb.EngineType.Pool else 1
            if si is not None and len(si.on_wait) > keep:
                waits = list(si.on_wait)
                moved = waits[:-keep] if keep else waits
                kept = waits[-keep:] if keep else []
                for w in moved:
                    ctr += 1
                    ev = mb.InstEventSemaphore(
                        name=f"zz_waitsplit_{ctr}", ins=[], outs=[])
                    ev.engine = inst.engine
                    ev.sync_info = mb.SyncInfo(on_wait=[w], on_update=[])
                    out.append(ev)
                inst.sync_info = mb.SyncInfo(
                    on_wait=kept, on_update=list(si.on_update))
                changed = True
            out.append(inst)
        if changed:
            bb.instructions = out


def _build_bass():
    import concourse.bass as bass
    import concourse.mybir as mybir
    from concourse.bass_types import AP
    from concourse.tile import TileContext

    F32 = mybir.dt.float32
    BF16 = mybir.dt.bfloat16
    Alu = mybir.AluOpType

    nc = bass.Bass("TRN2")
    k_d = nc.dram_tensor("kt", (NH, 128, 2 * L), BF16, kind="ExternalInput")
    q_d = nc.dram_tensor("qt", (NH, 128, 2 * L), BF16, kind="ExternalInput")
    v_d = nc.dram_tensor("v5", (NH, 128, NB * 325), BF16, kind="ExternalInput")
    m_d = nc.dram_tensor("msk", (NH, 128, 5 * LP), BF16, kind="ExternalInput")
    o_d = nc.dram_tensor("o", (NH, 128, NB * 320), BF16, kind="ExternalOutput")

    with TileContext(nc) as tc:
        with (
            tc.tile_pool(name="sb", bufs=2) as sb,
            tc.tile_pool(name="ps", bufs=3, space="PSUM") as ps,
            tc.tile_pool(name="ps_o", bufs=2, space="PSUM") as ps_o,
        ):
            def load_head(h, split_first):
                kT = sb.tile([128, 2 * L], BF16, tag="kT")
                qT = sb.tile([128, 2 * L], BF16, tag="qT")
                msk = sb.tile([128, 5 * LP], BF16, tag="msk")
                v5 = sb.tile([128, NB * 325], BF16, tag="v5")
                if split_first:
                    # load the partition halves needed by pair b=0 first so
                    # the first S matmul can start earlier
                    nc.sync.dma_start(kT[0:64, :], k_d[h, 0:64, :])
                    nc.sync.dma_start(qT[0:64, :], q_d[h, 0:64, :])
                    nc.sync.dma_start(msk[:, 0:LP], m_d[h, :, 0:LP])
                    nc.sync.dma_start(kT[64:128, :], k_d[h, 64:128, :])
                    nc.sync.dma_start(qT[64:128, :], q_d[h, 64:128, :])
                    for c in range(1, 5):
                        nc.sync.dma_start(msk[:, LP * c:LP * (c + 1)],
                                          m_d[h, :, LP * c:LP * (c + 1)])
                else:
                    nc.sync.dma_start(kT, k_d[h])
                    nc.sync.dma_start(qT, q_d[h])
                    nc.sync.dma_start(msk, m_d[h])
                nc.sync.dma_start(v5, v_d[h])
                o_sb = sb.tile([128, NB * 320], BF16, tag="o_sb")
                return dict(kT=kT, qT=qT, msk=msk, v5=v5, o_sb=o_sb, h=h)

            def s_chunk(R, b, c):
                pr = 64 * (b // 2)
                xo = L * (b % 2)
                cnt = CNT[c]
                sp = ps.tile([128, LP], F32, tag="sp")
                lhs = R["kT"][pr:pr + 64, xo + KS[c]:xo + KS[c] + cnt]
                nc.tensor.matmul(sp[0:cnt, 0:512], lhs,
                                 R["qT"][pr:pr + 64, xo:xo + 512],
                                 start=True, stop=True)
                nc.tensor.matmul(sp[0:cnt, 512:L], lhs,
                                 R["qT"][pr:pr + 64, xo + 512:xo + L],
                                 start=True, stop=True)
                return sp

            def av_group(Rj, j_b, a_t, o_ps, lc):
                # one PSUM accumulation group (all 5 k-chunks of one l-chunk);
                # c-order puts the late-masked chunks (1..3) last
                for idx, c in enumerate(AV_ORDER):
                    nc.tensor.matmul(
                        o_ps[0:LW[lc], 65 * lc:65 * lc + 65],
                        a_t[0:CNT[c], LP * c + 128 * lc:LP * c + 128 * lc + LW[lc]],
                        Rj["v5"][0:CNT[c], 325 * j_b + 65 * c:325 * j_b + 65 * c + 65],
                        start=(idx == 0), stop=(idx == 4))

            # per-slot state carried across the software pipeline
            pend = None            # (Rj, j_b, a_t_j, o_ps_j) awaiting finish

            pairs = [(h, b) for h in range(NH) for b in range(NB)]
            heads_loaded = [False] * NH
            R_by_head = {}

            def ensure_head(h):
                if not heads_loaded[h]:
                    R_by_head[h] = load_head(h, split_first=(h == 0))
                    heads_loaded[h] = True
                return R_by_head[h]

            ensure_head(0)

            for s, (h, b) in enumerate(pairs):
                R = R_by_head[h]
                # prefetch the next head's tensors a couple of slots early
                if b == 2 and h + 1 < NH:
                    ensure_head(h + 1)

                a_t = sb.tile([128, 5 * LP], BF16, tag="a_t")
                if pend is not None:
                    Rj, j_b, a_tj, o_psj = pend
                else:
                    Rj = j_b = a_tj = o_psj = None

                # ---- PE: S chunks of pair i interleaved with AV of pair j --
                sp0 = s_chunk(R, b, 0)
                sp1 = s_chunk(R, b, 1)
                if pend is not None:
                    av_group(Rj, j_b, a_tj, o_psj, 0)
                # DVE: fused copy+mask for chunk 0 straight from PSUM
                nc.vector.tensor_tensor(
                    out=a_t[0:121, 0:L], in0=sp0[0:121, 0:L],
                    in1=R["msk"][0:121, 0:L], op=Alu.mult)
                sp2 = s_chunk(R, b, 2)
                if pend is not None:
                    av_group(Rj, j_b, a_tj, o_psj, 1)
                # Act: copies for chunks 1..3
                nc.scalar.activation(a_t[0:120, LP:LP + L], sp1[0:120, 0:L],
                                     mybir.ActivationFunctionType.Copy)
                sp3 = s_chunk(R, b, 3)
                if pend is not None:
                    av_group(Rj, j_b, a_tj, o_psj, 2)
                nc.scalar.activation(a_t[0:120, 2 * LP:2 * LP + L],
                                     sp2[0:120, 0:L],
                                     mybir.ActivationFunctionType.Copy)
                sp4 = s_chunk(R, b, 4)
                if pend is not None:
                    av_group(Rj, j_b, a_tj, o_psj, 3)
                nc.scalar.activation(a_t[0:120, 3 * LP:3 * LP + L],
                                     sp3[0:120, 0:L],
                                     mybir.ActivationFunctionType.Copy)
                if pend is not None:
                    av_group(Rj, j_b, a_tj, o_psj, 4)
                # Act: chunk-4 head cols
                nc.scalar.activation(a_t[0:96, 4 * LP:4 * LP + A4],
                                     sp4[0:96, 0:A4],
                                     mybir.ActivationFunctionType.Copy)

                # DVE: chunk-4 tail cols fused from PSUM
                nc.vector.tensor_tensor(
                    out=a_t[0:96, 4 * LP + A4:4 * LP + L],
                    in0=sp4[0:96, A4:L],
                    in1=R["msk"][0:96, 4 * LP + A4:4 * LP + L], op=Alu.mult)

                # ---- finish pair j: recip + normalize (+ store) ----
                if pend is not None:
                    rz = sb.tile([128, 5], F32, tag="rz")
                    zin = o_psj[:, :].rearrange(
                        "p (c d) -> p c d", d=65)[:, :, 64:65]
                    nc.vector.reciprocal(
                        rz[:, :].rearrange("p (c d) -> p c d", d=1), zin)
                    in0 = o_psj[:, :].rearrange(
                        "p (c d) -> p c d", d=65)[:, :, 0:64]
                    in1 = AP(rz.tensor, 0, [[5, 128], [1, 5], [0, 64]])
                    nc.vector.tensor_tensor(
                        out=Rj["o_sb"][:, 320 * j_b:320 * j_b + 320].rearrange(
                            "p (c d) -> p c d", d=64),
                        in0=in0, in1=in1, op=Alu.mult)
                    nc.sync.dma_start(
                        o_d[Rj["h"], :, 320 * j_b:320 * j_b + 320],
                        Rj["o_sb"][:, 320 * j_b:320 * j_b + 320])

                # ---- DVE + Pool: mask multiplies for chunks 1..3 ----
                io_h = a_t[0:120, LP:4 * LP].rearrange(
                    "p (c l) -> p c l", l=LP)[:, :, 0:L - PC]
                mk_h = R["msk"][0:120, LP:4 * LP].rearrange(
                    "p (c l) -> p c l", l=LP)[:, :, 0:L - PC]
                nc.vector.tensor_tensor(out=io_h, in0=io_h, in1=mk_h,
                                        op=Alu.mult)
                for c in range(1, 4):
                    io_t = a_t[0:120, LP * c + L - PC:LP * c + L]
                    nc.gpsimd.tensor_tensor(
                        out=io_t, in0=io_t,
                        in1=R["msk"][0:120, LP * c + L - PC:LP * c + L],
                        op=Alu.mult)
                # DVE: chunk-4 head cols multiply
                io4 = a_t[0:96, 4 * LP:4 * LP + A4]
                nc.vector.tensor_tensor(
                    out=io4, in0=io4, in1=R["msk"][0:96, 4 * LP:4 * LP + A4],
                    op=Alu.mult)

                o_ps = ps_o.tile([128, 325], F32, tag="o_ps")
                pend = (R, b, a_t, o_ps)

            # ---- drain: AV + finish for the last pair ----
            Rj, j_b, a_tj, o_psj = pend
            for lc in range(5):
                av_group(Rj, j_b, a_tj, o_psj, lc)
            rz = sb.tile([128, 5], F32, tag="rz")
            zin = o_psj[:, :].rearrange("p (c d) -> p c d", d=65)[:, :, 64:65]
            nc.vector.reciprocal(
                rz[:, :].rearrange("p (c d) -> p c d", d=1), zin)
            in0 = o_psj[:, :].rearrange("p (c d) -> p c d", d=65)[:, :, 0:64]
            in1 = AP(rz.tensor, 0, [[5, 128], [1, 5], [0, 64]])
            nc.vector.tensor_tensor(
                out=Rj["o_sb"][:, 320 * j_b:320 * j_b + 320].rearrange(
                    "p (c d) -> p c d", d=64),
                in0=in0, in1=in1, op=Alu.mult)
            nc.sync.dma_start(
                o_d[Rj["h"], :, 320 * j_b:320 * j_b + 320],
                Rj["o_sb"][:, 320 * j_b:320 * j_b + 320])

    _split_excess_waits(nc)
    return nc


def _get_nc():
    if "nc" not in _CACHE:
        _CACHE["nc"] = _build_bass()
    return _CACHE["nc"]


def _dist_index():
    if "dist" not in _CACHE:
        gi = np.arange(NBX)
        gj = np.arange(NBY)
        di = (gi[:, None, None, None] - gi[None, None, :, None] + NBX) * 2 * NBY
        dj = gj[None, :, None, None] - gj[None, None, None, :] + NBY
        _CACHE["dist"] = (di + dj).reshape(NBX * NBY, NBX * NBY)
    return _CACHE["dist"]


def _host_shard(query, key, value, topological_params):
    """Build the 8 per-core input dicts (slicing / layout / packing)."""
    q = np.asarray(query, dtype=np.float32)
    k = np.asarray(key, dtype=np.float32)
    v = np.asarray(value, dtype=np.float32)
    p = np.asarray(topological_params, dtype=np.float32)

    qs = np.maximum(q * 0.125, 0.0)
    ks = np.maximum(k, 0.0)

    # per-head masks laid out like the device A tiles
    dist = _dist_index()
    absp = np.abs(p)
    msk_all = np.zeros((H, 128, 5 * LP), dtype=ml_dtypes.bfloat16)
    for h in range(H):
        M = np.ones((L, L), dtype=np.float32)
        M[1:, 1:] = np.take(absp[h], dist)      # [q_grid, k_grid]
        MT = M.T                                # [k, l]
        for c in range(5):
            n = CNT[c]
            msk_all[h, 0:n, LP * c:LP * c + L] = MT[KS[c]:KS[c] + n, :]

    def pack_T(x, bs, hs):
        # [4b, L, 3h, 64] -> [3h, 128p, 2*L]; p = d + 64*(b//2),
        # col = (b%2)*L + l
        t = x[bs, :, hs, :]                       # [4, L, 3, 64]
        t = t.transpose(2, 0, 3, 1)               # [3, 4, 64, L]
        t = t.reshape(3, 2, 2, 64, L)             # [3, bhi, blo, d, L]
        t = t.transpose(0, 1, 3, 2, 4)            # [3, bhi, d, blo, L]
        return np.ascontiguousarray(
            t.reshape(3, 128, 2 * L)).astype(ml_dtypes.bfloat16)

    in_maps = []
    for u in range(2):            # batch group
        for g in range(4):        # head group
            bs = slice(4 * u, 4 * u + 4)
            hs = slice(3 * g, 3 * g + 3)
            vs = v[bs, :, hs, :]                  # [4, L, 3, 64]
            v_r = np.zeros((3, 128, NB, 5, 65), np.float32)
            for c in range(5):
                n = CNT[c]
                blk = vs[:, KS[c]:KS[c] + n].transpose(2, 1, 0, 3)
                v_r[:, :n, :, c, 0:64] = blk
                v_r[:, :n, :, c, 64] = 1.0
            in_maps.append({
                "kt": pack_T(ks, bs, hs),
                "qt": pack_T(qs, bs, hs),
                "v5": np.ascontiguousarray(
                    v_r.reshape(3, 128, NB * 325)).astype(ml_dtypes.bfloat16),
                "msk": np.ascontiguousarray(msk_all[hs]),
            })
    return in_maps


def kernel(query, key, value, topological_params):
    from concourse import bass_utils
    nc = _get_nc()
    in_maps = _host_shard(query, key, value, topological_params)
    res = bass_utils.run_bass_kernel_spmd(nc, in_maps, core_ids=list(range(8)))
    out = np.empty((B, L, H, D), dtype=np.float32)
    for u in range(2):
        for g in range(4):
            o = res.results[4 * u + g]["o"]          # [3, 128, NB*320] bf16
            o = o.astype(np.float32).reshape(3, 128, NB, 5, 64)
            for lc in range(5):
                lw = LW[lc]
                blk = o[:, 0:lw, :, lc, :]           # [3, lw, 4, 64]
                out[4 * u:4 * u + 4, 128 * lc:128 * lc + lw,
                    3 * g:3 * g + 3, :] = blk.transpose(2, 1, 0, 3)
    return out


# revision 11
# speedup vs baseline: 1.4636x; 1.1713x over previous
"""Trainium2 Bass kernel for nn_MultiHeadDotProductAttention_24756191494231.

Masked (toeplitz-structured) linear attention:
    q = relu(query/8); k = relu(key)
    attn = (q @ k^T) * |toeplitz_mask| ; attn /= attn.sum(-1) ; out = attn @ v

Sharding: 8 cores = 2 batch-groups (4 batches) x 4 head-groups (3 heads).
Each core computes 12 (batch, head) pairs.

Host packs q/k (relu+scale folded, bf16), V with a ones column per k-chunk
(the AV matmul's 65th column then yields the row-sum Z), and the full
per-head |toeplitz| mask laid out exactly like the device A tile.

Device pipeline per (head, batch) pair, software-pipelined one slot deep
(masks of pair i overlap the AV matmuls of pair i-1 on the PE):
  S^T[k,l] = K'^T.T @ Q'^T          (bf16 matmuls into PSUM, 5 k-chunks)
  A[k,l]   = S^T * msk              split across engines per chunk
  O[l,:]   = A.T @ [V|1]            (bf16; ones column gives Z = row-sum)
  out      = O[:, :64] / Z          (reciprocal + multiply, bf16 DMA out)
"""
import sys

for _p in ("/opt/trn_rl_repo", "/root/.axon_site/_ro/trn_rl_repo"):
    if _p not in sys.path:
        sys.path.insert(0, _p)

import numpy as np
import ml_dtypes

NBX = NBY = 24
B, H, D = 8, 12, 64
L = NBX * NBY + 1          # 577
LP = 578                   # chunk stride in A/msk tiles (even => 4B aligned)
NB = 4                     # batches per core
NH = 3                     # heads per core
CNT = [121, 120, 120, 120, 96]       # k-chunk sizes (CLS + 24-aligned grid)
KS = [0, 121, 241, 361, 481]         # k-chunk starts
LW = [128, 128, 128, 128, 65]        # l-chunk sizes for the AV matmuls

# --- tuning knobs ---
PC = 380                   # tail cols of chunks 1..3 multiplied on GpSimd
A4 = 0                     # head cols of chunk 4 via ScalarE copy + DVE mult
AV_ORDER = [0, 4, 1, 2, 3]  # accumulation order of AV k-chunks
# PE-slot token sequence: Sc = S-matmul chunk c, Gk = AV group lc=k of the
# previous pair. Masks are emitted right after their S chunk.
SLOT_SEQ = ["S1", "S2", "S0", "S3", "G0", "S4", "G1", "G2", "G3", "G4"]
NORM_MODE = "dve"          # "dve" | "act_pool" | "act_dve"
AT_BUFS = 2
PC0 = 380                  # pool share for the first two (fill) slots
POOL_MERGE12 = False       # merge the c1+c2 pool tails into one op

_CACHE = {}


def _split_excess_waits(nc):
    """Walrus accepts at most ONE sync-wait per instruction (zero on
    Pool-engine ops). Move excess waits onto same-engine InstEventSemaphore
    instructions inserted immediately before the offending instruction."""
    import concourse.mybir as mb
    ctr = 0
    f = nc.m.functions[0]
    for bb in f.blocks:
        insts = list(bb.instructions)
        out = []
        changed = False
        for inst in insts:
            si = inst.sync_info
            keep = 0 if inst.engine == mb.EngineType.Pool else 1
            if si is not None and len(si.on_wait) > keep:
                waits = list(si.on_wait)
                moved = waits[:-keep] if keep else waits
                kept = waits[-keep:] if keep else []
                for w in moved:
                    ctr += 1
                    ev = mb.InstEventSemaphore(
                        name=f"zz_waitsplit_{ctr}", ins=[], outs=[])
                    ev.engine = inst.engine
                    ev.sync_info = mb.SyncInfo(on_wait=[w], on_update=[])
                    out.append(ev)
                inst.sync_info = mb.SyncInfo(
                    on_wait=kept, on_update=list(si.on_update))
                changed = True
            out.append(inst)
        if changed:
            bb.instructions = out


def _build_bass():
    import concourse.bass as bass
    import concourse.mybir as mybir
    from concourse.bass_types import AP
    from concourse.tile import TileContext

    F32 = mybir.dt.float32
    BF16 = mybir.dt.bfloat16
    Alu = mybir.AluOpType
    Act = mybir.ActivationFunctionType

    nc = bass.Bass("TRN2")
    qk_d = nc.dram_tensor("qk", (NH, 128, 4 * L), BF16, kind="ExternalInput")
    v_d = nc.dram_tensor("v5", (NH, 128, NB * 325), BF16, kind="ExternalInput")
    m_d = nc.dram_tensor("msk", (NH, 128, 5 * LP), BF16, kind="ExternalInput")
    o_d = nc.dram_tensor("o", (NH, 128, NB * 320), BF16, kind="ExternalOutput")

    with TileContext(nc) as tc:
        with (
            tc.tile_pool(name="sb", bufs=2) as sb,
            tc.tile_pool(name="ps", bufs=3, space="PSUM") as ps,
            tc.tile_pool(name="ps_o", bufs=2, space="PSUM") as ps_o,
        ):
            def load_head(h, split_first):
                qkT = sb.tile([128, 4 * L], BF16, tag="qkT")
                msk = sb.tile([128, 5 * LP], BF16, tag="msk")
                v5 = sb.tile([128, NB * 325], BF16, tag="v5")
                if split_first:
                    # partition half 0:64 first (pair b=0 only needs those
                    # rows), then all mask chunks (they gate the pair-0
                    # multiplies); rows 64:128 are not needed until pair 2
                    nc.sync.dma_start(qkT[0:64, :], qk_d[h, 0:64, :])
                    for c in range(5):
                        nc.sync.dma_start(msk[:, LP * c:LP * (c + 1)],
                                          m_d[h, :, LP * c:LP * (c + 1)])
                    nc.sync.dma_start(qkT[64:128, :], qk_d[h, 64:128, :])
                else:
                    nc.sync.dma_start(qkT, qk_d[h])
                    nc.sync.dma_start(msk, m_d[h])
                nc.sync.dma_start(v5, v_d[h])
                o_sb = sb.tile([128, NB * 320], BF16, tag="o_sb")
                return dict(kT=qkT[:, 0:2 * L], qT=qkT[:, 2 * L:4 * L],
                            msk=msk, v5=v5, o_sb=o_sb, h=h)

            def s_chunk(R, b, c):
                pr = 64 * (b // 2)
                xo = L * (b % 2)
                cnt = CNT[c]
                sp = ps.tile([128, LP], F32, tag="sp")
                lhs = R["kT"][pr:pr + 64, xo + KS[c]:xo + KS[c] + cnt]
                nc.tensor.matmul(sp[0:cnt, 0:512], lhs,
                                 R["qT"][pr:pr + 64, xo:xo + 512],
                                 start=True, stop=True)
                nc.tensor.matmul(sp[0:cnt, 512:L], lhs,
                                 R["qT"][pr:pr + 64, xo + 512:xo + L],
                                 start=True, stop=True)
                return sp

            def mask_chunk(R, a_t, sp, c):
                # emit the PSUM->SBUF path for chunk c right after its S
                if c == 0:
                    nc.vector.tensor_tensor(
                        out=a_t[0:121, 0:L], in0=sp[0:121, 0:L],
                        in1=R["msk"][0:121, 0:L], op=Alu.mult)
                elif c in (1, 2, 3):
                    nc.scalar.activation(a_t[0:120, LP * c:LP * c + L],
                                         sp[0:120, 0:L], Act.Copy)
                else:
                    if A4 > 0:
                        nc.scalar.activation(a_t[0:96, 4 * LP:4 * LP + A4],
                                             sp[0:96, 0:A4], Act.Copy)
                    nc.vector.tensor_tensor(
                        out=a_t[0:96, 4 * LP + A4:4 * LP + L],
                        in0=sp[0:96, A4:L],
                        in1=R["msk"][0:96, 4 * LP + A4:4 * LP + L],
                        op=Alu.mult)

            def av_group(Rj, j_b, a_t, o_ps, lc):
                for idx, c in enumerate(AV_ORDER):
                    nc.tensor.matmul(
                        o_ps[0:LW[lc], 65 * lc:65 * lc + 65],
                        a_t[0:CNT[c], LP * c + 128 * lc:LP * c + 128 * lc + LW[lc]],
                        Rj["v5"][0:CNT[c], 325 * j_b + 65 * c:325 * j_b + 65 * c + 65],
                        start=(idx == 0), stop=(idx == 4))

            def late_mults(R, a_t, pc):
                # mask multiplies for the Act-copied chunks of pair i
                io_h = a_t[0:120, LP:4 * LP].rearrange(
                    "p (c l) -> p c l", l=LP)[:, :, 0:L - pc]
                mk_h = R["msk"][0:120, LP:4 * LP].rearrange(
                    "p (c l) -> p c l", l=LP)[:, :, 0:L - pc]
                nc.vector.tensor_tensor(out=io_h, in0=io_h, in1=mk_h,
                                        op=Alu.mult)
                if pc > 0:
                    if POOL_MERGE12:
                        io12 = a_t[0:120, LP:3 * LP].rearrange(
                            "p (c l) -> p c l", l=LP)[:, :, L - pc:L]
                        mk12 = R["msk"][0:120, LP:3 * LP].rearrange(
                            "p (c l) -> p c l", l=LP)[:, :, L - pc:L]
                        nc.gpsimd.tensor_tensor(out=io12, in0=io12, in1=mk12,
                                                op=Alu.mult)
                        cs = (3,)
                    else:
                        cs = (1, 2, 3)
                    for c in cs:
                        io_t = a_t[0:120, LP * c + L - pc:LP * c + L]
                        nc.gpsimd.tensor_tensor(
                            out=io_t, in0=io_t,
                            in1=R["msk"][0:120, LP * c + L - pc:LP * c + L],
                            op=Alu.mult)
                if A4 > 0:
                    io4 = a_t[0:96, 4 * LP:4 * LP + A4]
                    nc.vector.tensor_tensor(
                        out=io4, in0=io4,
                        in1=R["msk"][0:96, 4 * LP:4 * LP + A4], op=Alu.mult)

            def finish_pair(Rj, j_b, o_psj, last):
                rz = sb.tile([128, 5], F32, tag="rz")
                zin = o_psj[:, :].rearrange(
                    "p (c d) -> p c d", d=65)[:, :, 64:65]
                nc.vector.reciprocal(
                    rz[:, :].rearrange("p (c d) -> p c d", d=1), zin)
                in0 = o_psj[:, :].rearrange(
                    "p (c d) -> p c d", d=65)[:, :, 0:64]
                rzb = AP(rz.tensor, 0, [[5, 128], [1, 5], [0, 64]])
                out_ap = Rj["o_sb"][:, 320 * j_b:320 * j_b + 320].rearrange(
                    "p (c d) -> p c d", d=64)
                if NORM_MODE in ("act_pool", "act_dve") and not last:
                    o_c = sb.tile([128, 320], BF16, tag="o_c")
                    nc.scalar.activation(
                        o_c[:, :].rearrange("p (c d) -> p c d", d=64), in0,
                        Act.Copy)
                    eng = nc.gpsimd if NORM_MODE == "act_pool" else nc.vector
                    eng.tensor_tensor(
                        out=out_ap,
                        in0=o_c[:, :].rearrange("p (c d) -> p c d", d=64),
                        in1=rzb, op=Alu.mult)
                else:
                    nc.vector.tensor_tensor(out=out_ap, in0=in0, in1=rzb,
                                            op=Alu.mult)
                nc.sync.dma_start(
                    o_d[Rj["h"], :, 320 * j_b:320 * j_b + 320],
                    Rj["o_sb"][:, 320 * j_b:320 * j_b + 320])

            pend = None            # (Rj, j_b, a_t_j, o_ps_j)

            pairs = [(h, b) for h in range(NH) for b in range(NB)]
            heads_loaded = [False] * NH
            R_by_head = {}

            def ensure_head(h):
                if not heads_loaded[h]:
                    R_by_head[h] = load_head(h, split_first=(h == 0))
                    heads_loaded[h] = True
                return R_by_head[h]

            ensure_head(0)

            for s, (h, b) in enumerate(pairs):
                R = R_by_head[h]
                if b == 2 and h + 1 < NH:
                    ensure_head(h + 1)

                a_t = sb.tile([128, 5 * LP], BF16, tag="a_t", bufs=AT_BUFS)
                if pend is not None:
                    Rj, j_b, a_tj, o_psj = pend

                for tok in SLOT_SEQ:
                    if tok[0] == "S":
                        c = int(tok[1])
                        sp = s_chunk(R, b, c)
                        mask_chunk(R, a_t, sp, c)
                    else:
                        if pend is not None:
                            av_group(Rj, j_b, a_tj, o_psj, int(tok[1]))

                if pend is not None:
                    finish_pair(Rj, j_b, o_psj, last=False)

                late_mults(R, a_t, PC0 if s < 2 else PC)

                o_ps = ps_o.tile([128, 325], F32, tag="o_ps")
                pend = (R, b, a_t, o_ps)

            # ---- drain ----
            Rj, j_b, a_tj, o_psj = pend
            for lc in range(5):
                av_group(Rj, j_b, a_tj, o_psj, lc)
            finish_pair(Rj, j_b, o_psj, last=True)

    _split_excess_waits(nc)
    return nc


def _get_nc():
    if "nc" not in _CACHE:
        _CACHE["nc"] = _build_bass()
    return _CACHE["nc"]


def _dist_index():
    if "dist" not in _CACHE:
        gi = np.arange(NBX)
        gj = np.arange(NBY)
        di = (gi[:, None, None, None] - gi[None, None, :, None] + NBX) * 2 * NBY
        dj = gj[None, :, None, None] - gj[None, None, None, :] + NBY
        _CACHE["dist"] = (di + dj).reshape(NBX * NBY, NBX * NBY)
    return _CACHE["dist"]


def _host_shard(query, key, value, topological_params):
    """Build the 8 per-core input dicts (slicing / layout / packing)."""
    q = np.asarray(query, dtype=np.float32)
    k = np.asarray(key, dtype=np.float32)
    v = np.asarray(value, dtype=np.float32)
    p = np.asarray(topological_params, dtype=np.float32)

    qs = np.maximum(q * 0.125, 0.0)
    ks = np.maximum(k, 0.0)

    dist = _dist_index()
    absp = np.abs(p)
    msk_all = np.zeros((H, 128, 5 * LP), dtype=ml_dtypes.bfloat16)
    for h in range(H):
        M = np.ones((L, L), dtype=np.float32)
        M[1:, 1:] = np.take(absp[h], dist)      # [q_grid, k_grid]
        MT = M.T                                # [k, l]
        for c in range(5):
            n = CNT[c]
            msk_all[h, 0:n, LP * c:LP * c + L] = MT[KS[c]:KS[c] + n, :]

    def pack_T(x, bs, hs):
        t = x[bs, :, hs, :]                       # [4, L, 3, 64]
        t = t.transpose(2, 0, 3, 1)               # [3, 4, 64, L]
        t = t.reshape(3, 2, 2, 64, L)             # [3, bhi, blo, d, L]
        t = t.transpose(0, 1, 3, 2, 4)            # [3, bhi, d, blo, L]
        return np.ascontiguousarray(
            t.reshape(3, 128, 2 * L)).astype(ml_dtypes.bfloat16)

    in_maps = []
    for u in range(2):            # batch group
        for g in range(4):        # head group
            bs = slice(4 * u, 4 * u + 4)
            hs = slice(3 * g, 3 * g + 3)
            vs = v[bs, :, hs, :]                  # [4, L, 3, 64]
            v_r = np.zeros((3, 128, NB, 5, 65), np.float32)
            for c in range(5):
                n = CNT[c]
                blk = vs[:, KS[c]:KS[c] + n].transpose(2, 1, 0, 3)
                v_r[:, :n, :, c, 0:64] = blk
                v_r[:, :n, :, c, 64] = 1.0
            in_maps.append({
                "qk": np.ascontiguousarray(np.concatenate(
                    [pack_T(ks, bs, hs), pack_T(qs, bs, hs)], axis=2)),
                "v5": np.ascontiguousarray(
                    v_r.reshape(3, 128, NB * 325)).astype(ml_dtypes.bfloat16),
                "msk": np.ascontiguousarray(msk_all[hs]),
            })
    return in_maps


def kernel(query, key, value, topological_params):
    from concourse import bass_utils
    nc = _get_nc()
    in_maps = _host_shard(query, key, value, topological_params)
    res = bass_utils.run_bass_kernel_spmd(nc, in_maps, core_ids=list(range(8)))
    out = np.empty((B, L, H, D), dtype=np.float32)
    for u in range(2):
        for g in range(4):
            o = res.results[4 * u + g]["o"]          # [3, 128, NB*320] bf16
            o = o.astype(np.float32).reshape(3, 128, NB, 5, 64)
            for lc in range(5):
                lw = LW[lc]
                blk = o[:, 0:lw, :, lc, :]           # [3, lw, 4, 64]
                out[4 * u:4 * u + 4, 128 * lc:128 * lc + lw,
                    3 * g:3 * g + 3, :] = blk.transpose(2, 1, 0, 3)
    return out
